# revision 12
# baseline (speedup 1.0000x reference)
"""AdaptiveBlockSelfAttention Trainium2 kernel (8 NeuronCores), fp8 version.

Math (per batch b, channel c, blocked layout; block index n, pixel p):
  Q/K/V = 1x1 conv of x (contract 192 ch), computed in fp8e4 DoubleRow
          matmuls (2 k-tiles of 96+bias row), weights pre-scaled by 32.
  T = K^T Q (contract n=256 as 2 k-tiles of 128, fp8 DoubleRow)
  E = exp(T/sqrt(C) - 2.5)  (shift cancels in the softmax ratio)
  U = E^T V (fp8 DR), denom = E^T 1 (fp8 DR, free-size-1 matmul)
  O = U * (1/denom), spilled as fp8.
  FFN (bf16): x1 = x + O; y = Wf2 gelu(Wf1 x1 + b1) + b2.
  Final residual out = x + O + y is applied on the HOST (x, O, y all
  available host-side), so the device never reloads x in full precision
  for the output add.

Sharding: core k = (b = k//2, h = k%2). Attention: 96 channels x full
image. FFN: all 192 channels x own token half. O halves exchanged with
chunked 2-core AllGathers overlapped with attention.

Token permutation: Q/K/V internal spill order pairs blocks (nm, nm+128)
so every DMA moves >=512B contiguous runs. O/y/x16 use natural blocked
order.
"""
import os
os.environ.setdefault("MYCRO_LOCAL_CACHE", "1")
import numpy as np
import ml_dtypes
import concourse.bass as bass
import concourse.bacc as bacc
import concourse.tile as tile
import concourse.mybir as mybir
from concourse.bass_utils import run_bass_kernel_spmd

F32 = mybir.dt.float32
BF16 = mybir.dt.bfloat16
FP8 = mybir.dt.float8e4
U32 = mybir.dt.uint32
AF = mybir.ActivationFunctionType
DR = mybir.MatmulPerfMode.DoubleRow

B, C, H, W = 4, 192, 256, 256
NPIX = H * W              # 65536 tokens per image
CH = C // 2               # 96 channels per core
HID = 384
HALF = NPIX // 2          # 32768 tokens per half
SCALE = 1.0 / float(np.sqrt(C))
ESHIFT = -2.5             # exp(T*SCALE + ESHIFT); cancels in ratio
WSCALE = 32.0             # QKV weights pre-scaled by 32 (fp8 subnormals)
NCHUNK = 4
CCH = CH // NCHUNK        # 48 channels per AllGather chunk
P1T = 2048                # P1 tokens per iter
P3T = 2048                # P3 tokens per iter

_NC_CACHE = {}


def build_nc(sim=False):
    nc = bacc.Bacc("TRN2", target_bir_lowering=False, debug=False,
                   num_devices=1 if sim else 8)
    # inputs
    x8 = nc.dram_tensor("x8", [2 * 97, NPIX], FP8, kind="ExternalInput")
    x16 = nc.dram_tensor("x16", [C, HALF], BF16, kind="ExternalInput")
    wq8 = nc.dram_tensor("wq8", [97, 2 * CH], FP8, kind="ExternalInput")
    wk8 = nc.dram_tensor("wk8", [97, 2 * CH], FP8, kind="ExternalInput")
    wv8 = nc.dram_tensor("wv8", [97, 2 * CH], FP8, kind="ExternalInput")
    wf1 = nc.dram_tensor("wf1", [C, HID], BF16, kind="ExternalInput")
    bf1c = nc.dram_tensor("bf1c", [HID, 1], F32, kind="ExternalInput")
    wf2 = nc.dram_tensor("wf2", [HID, C], BF16, kind="ExternalInput")
    dyn = nc.dram_tensor("dyn", [1, 4], U32, kind="ExternalInput")
    # outputs
    y16 = nc.dram_tensor("y16", [C, HALF], BF16, kind="ExternalOutput")
    o_own = nc.dram_tensor("o_own", [CH, HALF], FP8, kind="ExternalOutput")
    o_snd = nc.dram_tensor("o_snd", [CH, HALF], FP8, kind="ExternalOutput")
    # internal
    og = nc.dram_tensor("og", [NCHUNK * 2 * CCH, HALF], FP8)
    os_t = nc.dram_tensor("os", [1, 2 * CH * HALF], FP8)
    qkvs = nc.dram_tensor("qkvs", [CH, 3 * NPIX], FP8)

    x8v = x8.ap().rearrange("(j c) t -> c j t", j=2)
    qkv_w = qkvs.ap().rearrange("c (s t) -> c s t", s=3)
    qkv_r = qkvs.ap().rearrange("c (s n i p) -> n c s i p", s=3, i=2, p=256)
    os_r = os_t.ap().rearrange("o (r c t) -> (o r c) t", r=2, t=HALF)
    x16v = x16.ap().rearrange("(u c) t -> c u t", u=2)
    y16v = y16.ap().rearrange("(u c) t -> c u t", u=2)
    og_f = og.ap()

    def with_track(a, off):
        return bass.AP(tensor=a.tensor, offset=a.offset, ap=a.ap,
                       const_val=a.const_val,
                       runtime_checks=a.runtime_checks,
                       dep_tracking_offset=off)

    with tile.TileContext(nc) as tc:
        with tc.tile_pool(name="wpool", bufs=1) as wp:
            w8 = {}
            for nm, wt in (("q", wq8), ("k", wk8), ("v", wv8)):
                t = wp.tile([97, 2 * CH], FP8, name=f"w8{nm}", tag=f"w8{nm}")
                nc.sync.dma_start(t[:], wt.ap()[:, :])
                w8[nm] = t[:].rearrange("c (j m) -> c j m", j=2)
            wf1_t = []
            for u in range(2):
                t = wp.tile([CH, HID], BF16, name=f"wf1{u}", tag=f"wf1{u}")
                nc.sync.dma_start(t[:], wf1.ap()[u * CH:(u + 1) * CH, :])
                wf1_t.append(t)
            wf2_h = []
            for hc in range(3):
                t = wp.tile([128, C], BF16, name=f"wf2{hc}", tag=f"wf2{hc}")
                nc.sync.dma_start(t[:], wf2.ap()[hc * 128:(hc + 1) * 128, :])
                wf2_h.append(t)
            bf1_t = []
            for hc in range(3):
                t = wp.tile([128, 1], F32, name=f"bf1{hc}", tag=f"bf1{hc}")
                nc.sync.dma_start(t[:], bf1c.ap()[hc * 128:(hc + 1) * 128, :])
                bf1_t.append(t)
            ones2 = wp.tile([128, 2], FP8, name="ones2", tag="ones2")
            nc.vector.memset(ones2[:], 1.0)
            esh_t = wp.tile([128, 1], F32, name="esh", tag="esh")
            nc.vector.memset(esh_t[:], ESHIFT)
            dyn_sb = wp.tile([1, 4], U32, name="dyn", tag="dyn")
            nc.sync.dma_start(dyn_sb[:], dyn.ap()[:, :])
            o_m = [nc.values_load(dyn_sb[0:1, i:i + 1], min_val=0,
                                  max_val=CH * HALF,
                                  skip_runtime_bounds_check=True)
                   for i in range(2)]
            pb0 = nc.values_load(dyn_sb[0:1, 2:3], min_val=0, max_val=CCH,
                                 skip_runtime_bounds_check=True)

            # ---- phase 1: QKV projections (fp8 DoubleRow) ----
            cp_eng = [nc.vector, nc.scalar]
            with tc.tile_pool(name="px", bufs=3) as px, \
                 tc.tile_pool(name="pev", bufs=2) as pev, \
                 tc.tile_pool(name="psP", bufs=4, space="PSUM") as psP:
                nci = 0
                for t1 in range(NPIX // P1T):
                    xt = px.tile([97, 2, P1T], FP8, name="xt", tag="xt")
                    nc.sync.dma_start(xt[:], x8v[:, :, bass.ts(t1, P1T)])
                    comb = pev.tile([CH, 3, P1T], FP8, name="comb",
                                    tag="comb")
                    for hf in range(P1T // 1024):
                        for j, nm in enumerate(("q", "k", "v")):
                            ps = psP.tile([CH, 1024], F32, name="pp",
                                          tag="pp")
                            for q2 in range(2):
                                nc.tensor.matmul(
                                    ps[:, q2 * 512:(q2 + 1) * 512],
                                    w8[nm],
                                    xt[:, :, hf * 1024 + q2 * 512:
                                       hf * 1024 + (q2 + 1) * 512],
                                    start=True, stop=True, perf_mode=DR)
                            dst = comb[:, j, hf * 1024:(hf + 1) * 1024]
                            eng = cp_eng[nci % 2]
                            nci += 1
                            if eng is nc.scalar:
                                eng.activation(dst, ps[:], AF.Copy,
                                               scale=1.0 / WSCALE)
                            else:
                                eng.tensor_scalar_mul(dst, ps[:],
                                                      1.0 / WSCALE)
                    nc.scalar.dma_start(qkv_w[:, :, bass.ts(t1, P1T)],
                                        comb[:])

            # ---- phase 2: attention (fp8 DoubleRow) + AllGather ----
            ones2v = ones2[:].rearrange("q (j o) -> q j o", o=1)
            with tc.tile_pool(name="aq", bufs=3) as aq, \
                 tc.tile_pool(name="ao", bufs=2) as ao, \
                 tc.tile_pool(name="ar", bufs=4) as ar, \
                 tc.tile_pool(name="psT", bufs=2, space="PSUM") as psT, \
                 tc.tile_pool(name="psU", bufs=4, space="PSUM") as psU:
                for g4 in range(CH // 4):
                    c0 = g4 * 4
                    qv = aq.tile([128, 4, 3, 2, 256], FP8, name="qv",
                                 tag="qv")
                    nc.sync.dma_start(qv[:], qkv_r[:, c0:c0 + 4, :, :, :])
                    obt = ao.tile([128, 4, 2, 256], FP8, name="obt",
                                  tag="obt")
                    for up2 in range(2):
                        tps = psT.tile([128, 1024], F32, name="t", tag="t")
                        for c2 in range(2):
                            for j in range(2):
                                nc.tensor.matmul(
                                    tps[:, c2 * 512 + j * 256:
                                        c2 * 512 + (j + 1) * 256],
                                    qv[:, up2 * 2 + c2, 1, :,
                                       j * 128:(j + 1) * 128],
                                    qv[:, up2 * 2 + c2, 0, :, :],
                                    start=True, stop=True, perf_mode=DR)
                        esb = ar.tile([128, 1024], FP8, name="esb",
                                      tag="esb")
                        nc.scalar.activation(esb[:], tps[:], AF.Exp,
                                             bias=esh_t[:], scale=SCALE)
                        esbv = esb[:].rearrange("q (c j p) -> q c j p",
                                                c=2, j=2)
                        for c2 in range(2):
                            u = up2 * 2 + c2
                            ups = []
                            for m in range(2):
                                up = psU.tile([128, 512], F32, name="u",
                                              tag="u")
                                el = esbv[:, c2, :, m * 128:(m + 1) * 128]
                                nc.tensor.matmul(
                                    up[:, 0:256], el, qv[:, u, 2, :, :],
                                    start=True, stop=True, perf_mode=DR)
                                nc.tensor.matmul(
                                    up[:, 256:257], el, ones2v,
                                    start=True, stop=True, perf_mode=DR)
                                ups.append(up)
                            rc = ar.tile([128, 2], F32, name="rc", tag="rc")
                            for m in range(2):
                                nc.vector.reciprocal(
                                    rc[:, m:m + 1], ups[m][:, 256:257])
                            for m in range(2):
                                dst = obt[:, u, m, :]
                                if (u + m) % 4 == 3:
                                    nc.scalar.activation(
                                        dst, ups[m][:, 0:256], AF.Copy,
                                        scale=rc[:, m:m + 1])
                                else:
                                    nc.vector.tensor_scalar_mul(
                                        dst, ups[m][:, 0:256],
                                        rc[:, m:m + 1])
                    for m in range(2):
                        dst = os_t.ap()[0, bass.ds(o_m[m] + c0 * HALF,
                                                   4 * HALF)]
                        dst = dst.rearrange("(c n l) -> n c l", c=4, l=256)
                        dst = with_track(dst, c0 * HALF)
                        eng = nc.gpsimd if m == 0 else nc.sync
                        eng.dma_start(dst, obt[:, :, m, :])
                    # chunked exchange of the send region, plus the
                    # chunk's O output copies for the host
                    if (c0 + 4) % CCH == 0:
                        g = (c0 + 4) // CCH - 1
                        gs = slice(g * CCH, (g + 1) * CCH)
                        src = os_r[gs, :]
                        dst = og_f[g * 2 * CCH:(g + 1) * 2 * CCH, :]
                        if sim:
                            dv = dst.rearrange("(r c) t -> r c t", r=2)
                            nc.sync.dma_start(dv[0], src)
                            nc.sync.dma_start(dv[1], src)
                        else:
                            nc.gpsimd.collective_compute(
                                "AllGather", mybir.AluOpType.bypass,
                                replica_groups=[[0, 1], [2, 3], [4, 5],
                                                [6, 7]],
                                ins=[src], outs=[dst],
                            )
                        nc.sync.dma_start(
                            o_own.ap()[gs, :],
                            with_track(os_r[CH + g * CCH:
                                            CH + (g + 1) * CCH, :],
                                       g * CCH * HALF))
                        nc.gpsimd.dma_start(o_snd.ap()[gs, :],
                                            with_track(os_r[gs, :],
                                                       g * CCH * HALF))

            # ---- phase 3: FFN (bf16), y only; residual done on host ----
            with tc.tile_pool(name="fx", bufs=2) as fx, \
                 tc.tile_pool(name="fh", bufs=4) as fh, \
                 tc.tile_pool(name="fo", bufs=2) as fo, \
                 tc.tile_pool(name="psH", bufs=5, space="PSUM") as psH, \
                 tc.tile_pool(name="psY", bufs=2, space="PSUM") as psY:
                for t3 in range(HALF // P3T):
                    tsl = bass.ts(t3, P3T)
                    tx = fx.tile([CH, 2, P3T], BF16, name="tx", tag="tx")
                    nc.sync.dma_start(tx[:], x16v[:, :, tsl])
                    town = fx.tile([CH, P3T], FP8, name="town", tag="town")
                    nc.scalar.dma_start(
                        town[:], with_track(os_r[CH:2 * CH, tsl], t3 * P3T))
                    tpeer = fx.tile([CH, P3T], FP8, name="tpeer",
                                    tag="tpeer")
                    for gg in range(NCHUNK):
                        nc.scalar.dma_start(
                            tpeer[gg * CCH:(gg + 1) * CCH, :],
                            og_f[bass.ds(pb0 + gg * 2 * CCH, CCH), tsl])
                    x1f = fx.tile([CH, 2, P3T], BF16, name="x1f", tag="x1f")
                    nc.gpsimd.tensor_add(x1f[:, 0, :], tx[:, 0, :],
                                         town[:])
                    nc.gpsimd.tensor_add(x1f[:, 1, :], tx[:, 1, :],
                                         tpeer[:])
                    oo = fo.tile([CH, 2, P3T], BF16, name="oo", tag="oo")
                    for th in range(P3T // 512):
                        hsb = []
                        for hc in range(3):
                            hps = psH.tile([128, 512], F32, name="h",
                                           tag="h")
                            for u in range(2):
                                nc.tensor.matmul(
                                    hps[:],
                                    wf1_t[u][:, hc * 128:(hc + 1) * 128],
                                    x1f[:, u, th * 512:(th + 1) * 512],
                                    start=(u == 0), stop=(u == 1))
                            ht = fh.tile([128, 512], BF16, name=f"h{hc}",
                                         tag=f"h{hc}")
                            nc.scalar.activation(ht[:], hps[:], AF.Gelu,
                                                 bias=bf1_t[hc][:])
                            hsb.append(ht)
                        for cc in range(2):
                            yps = psY.tile([CH, 512], F32, name="y",
                                           tag="y")
                            for hc in range(3):
                                nc.tensor.matmul(
                                    yps[:],
                                    wf2_h[hc][:, cc * CH:(cc + 1) * CH],
                                    hsb[hc][:], start=(hc == 0),
                                    stop=(hc == 2))
                            dst = oo[:, cc, th * 512:(th + 1) * 512]
                            if (th + cc) % 2 == 0:
                                nc.vector.tensor_copy(dst, yps[:])
                            else:
                                nc.scalar.copy(dst, yps[:])
                    nc.sync.dma_start(y16v[:, :, tsl], oo[:])
    nc.compile()
    return nc


def _get_nc():
    if "nc" not in _NC_CACHE:
        _NC_CACHE["nc"] = build_nc()
    return _NC_CACHE["nc"]


def _block(x):
    """(B,C,256,256) -> (B,C,65536) blocked token order."""
    Bn, Cn = x.shape[0], x.shape[1]
    return (x.reshape(Bn, Cn, 16, 16, 16, 16)
            .transpose(0, 1, 2, 4, 3, 5)
            .reshape(Bn, Cn, NPIX))


def _unblock(y):
    """(B,C,65536) blocked -> (B,C,256,256)."""
    Bn, Cn = y.shape[0], y.shape[1]
    return (y.reshape(Bn, Cn, 16, 16, 16, 16)
            .transpose(0, 1, 2, 4, 3, 5)
            .reshape(Bn, Cn, H, W))


FP8NP = ml_dtypes.float8_e4m3


def prepare_in_maps(x, Wq, bq, Wk, bk, Wv, bv, Wf1, bf1, Wf2, bf2):
    xb = _block(np.asarray(x, np.float32))          # (B,192,65536)
    # qkv-permuted token order: blocks (nm, nm+128) interleaved
    xp = (xb.reshape(B, C, 2, 128, 256).transpose(0, 1, 3, 2, 4)
          .reshape(B, C, NPIX))
    wf1_f = np.asarray(Wf1, np.float32)
    wf2_f = np.asarray(Wf2, np.float32)
    bf1_in = np.asarray(bf1, np.float32).reshape(HID, 1)
    bf2_f = np.asarray(bf2, np.float32)
    in_maps = []
    for k in range(8):
        b, h = k // 2, k % 2
        perm = np.r_[np.arange(h * CH, (h + 1) * CH),
                     np.arange((1 - h) * CH, (2 - h) * CH)]
        # x8: [2*97, NPIX] fp8, permuted tokens, bias rows
        x8a = np.zeros((2 * 97, NPIX), np.float32)
        xpp = xp[b][perm]
        x8a[0:CH] = xpp[0:CH]
        x8a[96] = 1.0
        x8a[97:97 + CH] = xpp[CH:C]
        # x16: own token half, natural blocked order, bf16
        x16a = xb[b][perm][:, h * HALF:(h + 1) * HALF]
        m = {"x8": x8a.astype(FP8NP),
             "x16": x16a.astype(ml_dtypes.bfloat16),
             "wf1": np.ascontiguousarray(wf1_f[:, perm].T
                                         ).astype(ml_dtypes.bfloat16),
             "wf2": np.ascontiguousarray(wf2_f[perm].T
                                         ).astype(ml_dtypes.bfloat16),
             "bf1c": bf1_in,
             "dyn": np.array([[CH * HALF if h == 0 else 0,
                               CH * HALF if h == 1 else 0,
                               (1 - h) * CCH, 0]], np.uint32)}
        own = perm[:CH]
        for nm, Wm, bm in (("wq8", Wq, bq), ("wk8", Wk, bk),
                           ("wv8", Wv, bv)):
            Wl = np.asarray(Wm, np.float32)[own][:, perm]  # (96 out, 192 in)
            w8a = np.zeros((97, 2, CH), np.float32)
            for j in range(2):
                w8a[0:CH, j, :] = WSCALE * Wl[:, j * CH:(j + 1) * CH].T
            w8a[96, 0, :] = WSCALE * np.asarray(bm, np.float32)[own]
            m[nm] = w8a.reshape(97, 2 * CH).astype(FP8NP)
        in_maps.append(m)
    return in_maps


def run(in_maps, trace=False, **kw):
    nc = _get_nc()
    return run_bass_kernel_spmd(nc, in_maps, core_ids=list(range(8)),
                                trace=trace, **kw)


def assemble(results, x, bf2):
    """Host-side final residual: out = x + O + y + bf2."""
    bf2 = np.asarray(bf2, np.float32)
    xb = _block(np.asarray(x, np.float32))
    outb = np.zeros((B, C, NPIX), np.float32)
    for k in range(8):
        b, h = k // 2, k % 2
        perm = np.r_[np.arange(h * CH, (h + 1) * CH),
                     np.arange((1 - h) * CH, (2 - h) * CH)]
        r = results[k]
        # O own half from o_own (region1); sent half from o_snd (region0)
        outb[b, perm[:CH], h * HALF:(h + 1) * HALF] += \
            r["o_own"].astype(np.float32)
        outb[b, perm[:CH], (1 - h) * HALF:(2 - h) * HALF] += \
            r["o_snd"].astype(np.float32)
        # y for all 192 channels, own token half (bias applied here)
        outb[b, perm, h * HALF:(h + 1) * HALF] += \
            r["y16"].astype(np.float32) + bf2[perm][:, None]
    outb += xb
    return _unblock(outb)


def kernel(**inputs):
    in_maps = prepare_in_maps(**inputs)
    res = run(in_maps)
    return assemble(res.results, inputs["x"], inputs["bf2"])


# revision 13
# speedup vs baseline: 1.1109x; 1.1109x over previous
"""AdaptiveBlockSelfAttention Trainium2 kernel (8 NeuronCores), fp8 version.

Math (per batch b, channel c, blocked layout; block index n, pixel p):
  Q/K/V = 1x1 conv of x (contract 192 ch), computed in fp8e4 DoubleRow
          matmuls (2 k-tiles of 96+bias row), weights pre-scaled by 32.
  T = K^T Q (contract n=256 as 2 k-tiles of 128, fp8 DoubleRow)
  E = exp(T/sqrt(C) - 2.5)  (shift cancels in the softmax ratio)
  U = E^T V (fp8 DR), denom = E^T 1 (fp8 DR, free-size-1 matmul)
  O = U * (1/denom), spilled as fp8.
  FFN (bf16): x1 = x + O; y = Wf2 gelu(Wf1 x1 + b1) + b2.
  Final residual out = x + O + y is applied on the HOST (x, O, y all
  available host-side), so the device never reloads x in full precision
  for the output add.

Sharding: core k = (b = k//2, h = k%2). Attention: 96 channels x full
image. FFN: all 192 channels x own token half. O halves exchanged with
chunked 2-core AllGathers overlapped with attention.

Token permutation: Q/K/V internal spill order pairs blocks (nm, nm+128)
so every DMA moves >=512B contiguous runs. O/y/x16 use natural blocked
order.
"""
import os
os.environ.setdefault("MYCRO_LOCAL_CACHE", "1")
import numpy as np
import ml_dtypes
import concourse.bass as bass
import concourse.bacc as bacc
import concourse.tile as tile
import concourse.mybir as mybir
from concourse.bass_utils import run_bass_kernel_spmd

F32 = mybir.dt.float32
BF16 = mybir.dt.bfloat16
FP8 = mybir.dt.float8e4
U32 = mybir.dt.uint32
AF = mybir.ActivationFunctionType
DR = mybir.MatmulPerfMode.DoubleRow

B, C, H, W = 4, 192, 256, 256
NPIX = H * W              # 65536 tokens per image
CH = C // 2               # 96 channels per core
HID = 384
HALF = NPIX // 2          # 32768 tokens per half
SCALE = 1.0 / float(np.sqrt(C))
ESHIFT = -2.5             # exp(T*SCALE + ESHIFT); cancels in ratio
WSCALE = 32.0             # QKV weights pre-scaled by 32 (fp8 subnormals)
NCHUNK = 4
CCH = CH // NCHUNK        # 48 channels per AllGather chunk
P1T = 2048                # P1 tokens per iter
P3T = 2048                # P3 tokens per iter

_NC_CACHE = {}


def build_nc(sim=False):
    nc = bacc.Bacc("TRN2", target_bir_lowering=False, debug=False,
                   num_devices=1 if sim else 8)
    # inputs
    x8 = nc.dram_tensor("x8", [2 * 97, NPIX], FP8, kind="ExternalInput")
    x16 = nc.dram_tensor("x16", [C, HALF], BF16, kind="ExternalInput")
    wq8 = nc.dram_tensor("wq8", [97, 2 * CH], FP8, kind="ExternalInput")
    wk8 = nc.dram_tensor("wk8", [97, 2 * CH], FP8, kind="ExternalInput")
    wv8 = nc.dram_tensor("wv8", [97, 2 * CH], FP8, kind="ExternalInput")
    wf1 = nc.dram_tensor("wf1", [C, HID], BF16, kind="ExternalInput")
    bf1c = nc.dram_tensor("bf1c", [HID, 1], F32, kind="ExternalInput")
    wf2 = nc.dram_tensor("wf2", [HID, C], BF16, kind="ExternalInput")
    dyn = nc.dram_tensor("dyn", [1, 4], U32, kind="ExternalInput")
    # outputs
    y16 = nc.dram_tensor("y16", [C, HALF], BF16, kind="ExternalOutput")
    o_own = nc.dram_tensor("o_own", [CH, HALF], FP8, kind="ExternalOutput")
    o_snd = nc.dram_tensor("o_snd", [CH, HALF], FP8, kind="ExternalOutput")
    # internal
    og = nc.dram_tensor("og", [NCHUNK * 2 * CCH, HALF], FP8)
    os_t = nc.dram_tensor("os", [1, 2 * CH * HALF], FP8)
    qkvs = nc.dram_tensor("qkvs", [CH, 3 * NPIX], FP8)

    x8v = x8.ap().rearrange("(j c) t -> c j t", j=2)
    qkv_w = qkvs.ap().rearrange("c (s t) -> c s t", s=3)
    qkv_r = qkvs.ap().rearrange("c (s n i p) -> n c s i p", s=3, i=2, p=256)
    os_r = os_t.ap().rearrange("o (r c t) -> (o r c) t", r=2, t=HALF)
    x16v = x16.ap().rearrange("(u c) t -> c u t", u=2)
    y16v = y16.ap().rearrange("(u c) t -> c u t", u=2)
    og_f = og.ap()

    def with_track(a, off):
        return bass.AP(tensor=a.tensor, offset=a.offset, ap=a.ap,
                       const_val=a.const_val,
                       runtime_checks=a.runtime_checks,
                       dep_tracking_offset=off)

    with tile.TileContext(nc) as tc:
        with tc.tile_pool(name="wpool", bufs=1) as wp:
            w8 = {}
            for nm, wt in (("q", wq8), ("k", wk8), ("v", wv8)):
                t = wp.tile([97, 2 * CH], FP8, name=f"w8{nm}", tag=f"w8{nm}")
                nc.sync.dma_start(t[:], wt.ap()[:, :])
                w8[nm] = t[:].rearrange("c (j m) -> c j m", j=2)
            wf1_t = []
            for u in range(2):
                t = wp.tile([CH, HID], BF16, name=f"wf1{u}", tag=f"wf1{u}")
                nc.sync.dma_start(t[:], wf1.ap()[u * CH:(u + 1) * CH, :])
                wf1_t.append(t)
            wf2_h = []
            for hc in range(3):
                t = wp.tile([128, C], BF16, name=f"wf2{hc}", tag=f"wf2{hc}")
                nc.sync.dma_start(t[:], wf2.ap()[hc * 128:(hc + 1) * 128, :])
                wf2_h.append(t)
            bf1_t = []
            for hc in range(3):
                t = wp.tile([128, 1], F32, name=f"bf1{hc}", tag=f"bf1{hc}")
                nc.sync.dma_start(t[:], bf1c.ap()[hc * 128:(hc + 1) * 128, :])
                bf1_t.append(t)
            ones2 = wp.tile([128, 2], FP8, name="ones2", tag="ones2")
            nc.vector.memset(ones2[:], 1.0)
            esh_t = wp.tile([128, 1], F32, name="esh", tag="esh")
            nc.vector.memset(esh_t[:], ESHIFT)
            dyn_sb = wp.tile([1, 4], U32, name="dyn", tag="dyn")
            nc.sync.dma_start(dyn_sb[:], dyn.ap()[:, :])
            o_m = [nc.values_load(dyn_sb[0:1, i:i + 1], min_val=0,
                                  max_val=CH * HALF,
                                  skip_runtime_bounds_check=True)
                   for i in range(2)]
            pb0 = nc.values_load(dyn_sb[0:1, 2:3], min_val=0, max_val=CCH,
                                 skip_runtime_bounds_check=True)

            # ---- phase 1: QKV projections (fp8 DoubleRow) ----
            cp_eng = [nc.vector, nc.scalar]
            with tc.tile_pool(name="px", bufs=3) as px, \
                 tc.tile_pool(name="pev", bufs=2) as pev, \
                 tc.tile_pool(name="psP", bufs=4, space="PSUM") as psP:
                nci = 0
                for t1 in range(NPIX // P1T):
                    xt = px.tile([97, 2, P1T], FP8, name="xt", tag="xt")
                    nc.sync.dma_start(xt[:], x8v[:, :, bass.ts(t1, P1T)])
                    comb = pev.tile([CH, 3, P1T], FP8, name="comb",
                                    tag="comb")
                    for hf in range(P1T // 1024):
                        for j, nm in enumerate(("q", "k", "v")):
                            ps = psP.tile([CH, 1024], F32, name="pp",
                                          tag="pp")
                            for q2 in range(2):
                                nc.tensor.matmul(
                                    ps[:, q2 * 512:(q2 + 1) * 512],
                                    w8[nm],
                                    xt[:, :, hf * 1024 + q2 * 512:
                                       hf * 1024 + (q2 + 1) * 512],
                                    start=True, stop=True, perf_mode=DR)
                            dst = comb[:, j, hf * 1024:(hf + 1) * 1024]
                            eng = cp_eng[nci % 2]
                            nci += 1
                            if eng is nc.scalar:
                                eng.activation(dst, ps[:], AF.Copy,
                                               scale=1.0 / WSCALE)
                            else:
                                eng.tensor_scalar_mul(dst, ps[:],
                                                      1.0 / WSCALE)
                    nc.scalar.dma_start(qkv_w[:, :, bass.ts(t1, P1T)],
                                        comb[:])

            # ---- phase 2: attention (fp8 DoubleRow) + AllGather ----
            ones2v = ones2[:].rearrange("q (j o) -> q j o", o=1)
            with tc.tile_pool(name="aq", bufs=3) as aq, \
                 tc.tile_pool(name="ao", bufs=2) as ao, \
                 tc.tile_pool(name="ar", bufs=4) as ar, \
                 tc.tile_pool(name="psT", bufs=2, space="PSUM") as psT, \
                 tc.tile_pool(name="psU", bufs=4, space="PSUM") as psU:
                for g4 in range(CH // 4):
                    c0 = g4 * 4
                    qv = aq.tile([128, 4, 3, 2, 256], FP8, name="qv",
                                 tag="qv")
                    nc.sync.dma_start(qv[:], qkv_r[:, c0:c0 + 4, :, :, :])
                    obt = ao.tile([128, 4, 2, 256], FP8, name="obt",
                                  tag="obt")
                    for up2 in range(2):
                        tps = psT.tile([128, 1024], F32, name="t", tag="t")
                        for c2 in range(2):
                            for j in range(2):
                                nc.tensor.matmul(
                                    tps[:, c2 * 512 + j * 256:
                                        c2 * 512 + (j + 1) * 256],
                                    qv[:, up2 * 2 + c2, 1, :,
                                       j * 128:(j + 1) * 128],
                                    qv[:, up2 * 2 + c2, 0, :, :],
                                    start=True, stop=True, perf_mode=DR)
                        esb = ar.tile([128, 1024], FP8, name="esb",
                                      tag="esb")
                        nc.scalar.activation(esb[:], tps[:], AF.Exp,
                                             bias=esh_t[:], scale=SCALE)
                        esbv = esb[:].rearrange("q (c j p) -> q c j p",
                                                c=2, j=2)
                        for c2 in range(2):
                            u = up2 * 2 + c2
                            ups = []
                            for m in range(2):
                                up = psU.tile([128, 512], F32, name="u",
                                              tag="u")
                                el = esbv[:, c2, :, m * 128:(m + 1) * 128]
                                nc.tensor.matmul(
                                    up[:, 0:256], el, qv[:, u, 2, :, :],
                                    start=True, stop=True, perf_mode=DR)
                                nc.tensor.matmul(
                                    up[:, 256:257], el, ones2v,
                                    start=True, stop=True, perf_mode=DR)
                                ups.append(up)
                            rc = ar.tile([128, 2], F32, name="rc", tag="rc")
                            for m in range(2):
                                nc.vector.reciprocal(
                                    rc[:, m:m + 1], ups[m][:, 256:257])
                            for m in range(2):
                                dst = obt[:, u, m, :]
                                if (u + m) % 4 == 3:
                                    nc.scalar.activation(
                                        dst, ups[m][:, 0:256], AF.Copy,
                                        scale=rc[:, m:m + 1])
                                else:
                                    nc.vector.tensor_scalar_mul(
                                        dst, ups[m][:, 0:256],
                                        rc[:, m:m + 1])
                    for m in range(2):
                        dst = os_t.ap()[0, bass.ds(o_m[m] + c0 * HALF,
                                                   4 * HALF)]
                        dst = dst.rearrange("(c n l) -> n c l", c=4, l=256)
                        dst = with_track(dst, c0 * HALF)
                        eng = nc.gpsimd if m == 0 else nc.sync
                        eng.dma_start(dst, obt[:, :, m, :])
                    # chunked exchange of the send region, plus the
                    # chunk's O output copies for the host
                    if (c0 + 4) % CCH == 0:
                        g = (c0 + 4) // CCH - 1
                        gs = slice(g * CCH, (g + 1) * CCH)
                        src = os_r[gs, :]
                        dst = og_f[g * 2 * CCH:(g + 1) * 2 * CCH, :]
                        if sim:
                            dv = dst.rearrange("(r c) t -> r c t", r=2)
                            nc.sync.dma_start(dv[0], src)
                            nc.sync.dma_start(dv[1], src)
                        else:
                            nc.gpsimd.collective_compute(
                                "AllGather", mybir.AluOpType.bypass,
                                replica_groups=[[0, 1], [2, 3], [4, 5],
                                                [6, 7]],
                                ins=[src], outs=[dst],
                            )


            # ---- phase 3: FFN (bf16), y only; residual done on host ----
            with tc.tile_pool(name="fx", bufs=2) as fx, \
                 tc.tile_pool(name="fh", bufs=4) as fh, \
                 tc.tile_pool(name="fo", bufs=2) as fo, \
                 tc.tile_pool(name="psH", bufs=5, space="PSUM") as psH, \
                 tc.tile_pool(name="psY", bufs=2, space="PSUM") as psY:
                for t3 in range(HALF // P3T):
                    tsl = bass.ts(t3, P3T)
                    if t3 < NCHUNK:
                        g = t3
                        gs = slice(g * CCH, (g + 1) * CCH)
                        nc.gpsimd.dma_start(
                            o_own.ap()[gs, :],
                            with_track(os_r[CH + g * CCH:
                                            CH + (g + 1) * CCH, :],
                                       g * CCH * HALF))
                        nc.gpsimd.dma_start(o_snd.ap()[gs, :],
                                            with_track(os_r[gs, :],
                                                       g * CCH * HALF))
                    tx = fx.tile([CH, 2, P3T], BF16, name="tx", tag="tx")
                    nc.sync.dma_start(tx[:], x16v[:, :, tsl])
                    town = fx.tile([CH, P3T], FP8, name="town", tag="town")
                    nc.scalar.dma_start(
                        town[:], with_track(os_r[CH:2 * CH, tsl], t3 * P3T))
                    tpeer = fx.tile([CH, P3T], FP8, name="tpeer",
                                    tag="tpeer")
                    for gg in range(NCHUNK):
                        nc.scalar.dma_start(
                            tpeer[gg * CCH:(gg + 1) * CCH, :],
                            og_f[bass.ds(pb0 + gg * 2 * CCH, CCH), tsl])
                    x1f = fx.tile([CH, 2, P3T], BF16, name="x1f", tag="x1f")
                    nc.vector.tensor_add(x1f[:, 0, :], tx[:, 0, :],
                                         town[:])
                    nc.vector.tensor_add(x1f[:, 1, :], tx[:, 1, :],
                                         tpeer[:])
                    oo = fo.tile([CH, 2, P3T], BF16, name="oo", tag="oo")
                    for th in range(P3T // 512):
                        hsb = []
                        for hc in range(3):
                            hps = psH.tile([128, 512], F32, name="h",
                                           tag="h")
                            for u in range(2):
                                nc.tensor.matmul(
                                    hps[:],
                                    wf1_t[u][:, hc * 128:(hc + 1) * 128],
                                    x1f[:, u, th * 512:(th + 1) * 512],
                                    start=(u == 0), stop=(u == 1))
                            ht = fh.tile([128, 512], BF16, name=f"h{hc}",
                                         tag=f"h{hc}")
                            nc.scalar.activation(ht[:], hps[:], AF.Gelu,
                                                 bias=bf1_t[hc][:])
                            hsb.append(ht)
                        for cc in range(2):
                            yps = psY.tile([CH, 512], F32, name="y",
                                           tag="y")
                            for hc in range(3):
                                nc.tensor.matmul(
                                    yps[:],
                                    wf2_h[hc][:, cc * CH:(cc + 1) * CH],
                                    hsb[hc][:], start=(hc == 0),
                                    stop=(hc == 2))
                            dst = oo[:, cc, th * 512:(th + 1) * 512]
                            nc.vector.tensor_copy(dst, yps[:])
                    nc.sync.dma_start(y16v[:, :, tsl], oo[:])
    nc.compile()
    return nc


def _get_nc():
    if "nc" not in _NC_CACHE:
        _NC_CACHE["nc"] = build_nc()
    return _NC_CACHE["nc"]


def _block(x):
    """(B,C,256,256) -> (B,C,65536) blocked token order."""
    Bn, Cn = x.shape[0], x.shape[1]
    return (x.reshape(Bn, Cn, 16, 16, 16, 16)
            .transpose(0, 1, 2, 4, 3, 5)
            .reshape(Bn, Cn, NPIX))


def _unblock(y):
    """(B,C,65536) blocked -> (B,C,256,256)."""
    Bn, Cn = y.shape[0], y.shape[1]
    return (y.reshape(Bn, Cn, 16, 16, 16, 16)
            .transpose(0, 1, 2, 4, 3, 5)
            .reshape(Bn, Cn, H, W))


FP8NP = ml_dtypes.float8_e4m3


def prepare_in_maps(x, Wq, bq, Wk, bk, Wv, bv, Wf1, bf1, Wf2, bf2):
    xb = _block(np.asarray(x, np.float32))          # (B,192,65536)
    # qkv-permuted token order: blocks (nm, nm+128) interleaved
    xp = (xb.reshape(B, C, 2, 128, 256).transpose(0, 1, 3, 2, 4)
          .reshape(B, C, NPIX))
    wf1_f = np.asarray(Wf1, np.float32)
    wf2_f = np.asarray(Wf2, np.float32)
    bf1_in = np.asarray(bf1, np.float32).reshape(HID, 1)
    bf2_f = np.asarray(bf2, np.float32)
    in_maps = []
    for k in range(8):
        b, h = k // 2, k % 2
        perm = np.r_[np.arange(h * CH, (h + 1) * CH),
                     np.arange((1 - h) * CH, (2 - h) * CH)]
        # x8: [2*97, NPIX] fp8, permuted tokens, bias rows
        x8a = np.zeros((2 * 97, NPIX), np.float32)
        xpp = xp[b][perm]
        x8a[0:CH] = xpp[0:CH]
        x8a[96] = 1.0
        x8a[97:97 + CH] = xpp[CH:C]
        # x16: own token half, natural blocked order, bf16
        x16a = xb[b][perm][:, h * HALF:(h + 1) * HALF]
        m = {"x8": x8a.astype(FP8NP),
             "x16": x16a.astype(ml_dtypes.bfloat16),
             "wf1": np.ascontiguousarray(wf1_f[:, perm].T
                                         ).astype(ml_dtypes.bfloat16),
             "wf2": np.ascontiguousarray(wf2_f[perm].T
                                         ).astype(ml_dtypes.bfloat16),
             "bf1c": bf1_in,
             "dyn": np.array([[CH * HALF if h == 0 else 0,
                               CH * HALF if h == 1 else 0,
                               (1 - h) * CCH, 0]], np.uint32)}
        own = perm[:CH]
        for nm, Wm, bm in (("wq8", Wq, bq), ("wk8", Wk, bk),
                           ("wv8", Wv, bv)):
            Wl = np.asarray(Wm, np.float32)[own][:, perm]  # (96 out, 192 in)
            w8a = np.zeros((97, 2, CH), np.float32)
            for j in range(2):
                w8a[0:CH, j, :] = WSCALE * Wl[:, j * CH:(j + 1) * CH].T
            w8a[96, 0, :] = WSCALE * np.asarray(bm, np.float32)[own]
            m[nm] = w8a.reshape(97, 2 * CH).astype(FP8NP)
        in_maps.append(m)
    return in_maps


def run(in_maps, trace=False, **kw):
    nc = _get_nc()
    return run_bass_kernel_spmd(nc, in_maps, core_ids=list(range(8)),
                                trace=trace, **kw)


def assemble(results, x, bf2):
    """Host-side final residual: out = x + O + y + bf2."""
    bf2 = np.asarray(bf2, np.float32)
    xb = _block(np.asarray(x, np.float32))
    outb = np.zeros((B, C, NPIX), np.float32)
    for k in range(8):
        b, h = k // 2, k % 2
        perm = np.r_[np.arange(h * CH, (h + 1) * CH),
                     np.arange((1 - h) * CH, (2 - h) * CH)]
        r = results[k]
        # O own half from o_own (region1); sent half from o_snd (region0)
        outb[b, perm[:CH], h * HALF:(h + 1) * HALF] += \
            r["o_own"].astype(np.float32)
        outb[b, perm[:CH], (1 - h) * HALF:(2 - h) * HALF] += \
            r["o_snd"].astype(np.float32)
        # y for all 192 channels, own token half (bias applied here)
        outb[b, perm, h * HALF:(h + 1) * HALF] += \
            r["y16"].astype(np.float32) + bf2[perm][:, None]
    outb += xb
    return _unblock(outb)


def kernel(**inputs):
    in_maps = prepare_in_maps(**inputs)
    res = run(in_maps)
    return assemble(res.results, inputs["x"], inputs["bf2"])
